# revision 8
# baseline (speedup 1.0000x reference)
"""Trainium2 Bass kernel v5 for 16-head causal self-attention (KaplanAttention).

Sharding (8 cores): core c handles batch b = c // 4 and head group g = c % 4
(heads 4g..4g+3); host sums the 4 partial output projections per batch.

v5 scheduling insight: exp on ScalarE (~72us) is 2x slower than the warm-clock
score matmuls it consumes (~29us), while ALL other PE work (projections, AV,
transposes, final projection, ~72us) almost exactly matches exp time. So the
emission interleaves at ~0.5us granularity: between every score matmul of
window st, weave an AV chain of window st-1 or a projection/final group.
PE then stays near-100% dense through the whole kernel, which also keeps the
HAM clock gate at 2.4 GHz (sparse PE re-throttles to 1.2 GHz within ~3.4us).

Other structure (from v2-v4): s-major streaming attention; AV flipped so the
softmax denominator lands per-partition (cheap reciprocal + tensor_scalar);
causal mask as a {0,1} DVE multiply on diagonal blocks; deferred PE
transposes bring output back to [d, s] for the final projection; weights
loaded before x; fp16 partial outputs summed on host in fp32.
"""

import numpy as np

from concourse import bass_utils, mybir, tile
from concourse import bacc

S = 2048
D = 1024
HPC = 4        # heads per core
DK = 64
DC = HPC * DK  # 256 d-columns per core
NCORES = 8
EC = D // 128  # 8 e-chunks
NJT = S // 128  # 16 j-tiles
NST = S // 512  # 4 s-windows of 512

FP16 = mybir.dt.float16
FP32 = mybir.dt.float32


def _build():
    nc = bacc.Bacc("TRN2", target_bir_lowering=False, debug=False)

    xT_d = nc.dram_tensor("xT", [D, S], FP16, kind="ExternalInput")
    wq_d = nc.dram_tensor("wqT", [D, DC], FP16, kind="ExternalInput")
    wk_d = nc.dram_tensor("wkT", [D, DC], FP16, kind="ExternalInput")
    wv_d = nc.dram_tensor("wvT", [D, DC], FP16, kind="ExternalInput")
    wo_d = nc.dram_tensor("woT", [DC, D], FP16, kind="ExternalInput")
    mask_d = nc.dram_tensor("mask", [128, 2 * 128], FP16, kind="ExternalInput")
    id_d = nc.dram_tensor("ident", [128, 128], FP16, kind="ExternalInput")
    out_d = nc.dram_tensor("out", [S, D], FP16, kind="ExternalOutput")

    with tile.TileContext(nc) as tc:
        with (
            tc.tile_pool(name="const", bufs=1) as const,
            tc.tile_pool(name="work", bufs=1) as work,
            tc.tile_pool(name="upool", bufs=2) as upool,
            tc.tile_pool(name="ospool", bufs=10) as ospool,
            tc.tile_pool(name="zpool", bufs=8) as zpool,
            tc.tile_pool(name="obpool", bufs=3) as obpool,
            tc.tile_pool(name="psBig", bufs=2, space="PSUM") as psBig,
            tc.tile_pool(name="psSm", bufs=4, space="PSUM") as psSm,
        ):
            # ---- load inputs (weights early so compute starts right away) ----
            xT = const.tile([128, EC, S], FP16)
            wq = const.tile([128, EC, DC], FP16)
            wk = const.tile([128, EC, DC], FP16)
            wv = const.tile([128, EC, DC], FP16)
            wo = const.tile([128, 2, D], FP16)
            maskD = const.tile([128, 2, 128], FP16)
            ident = const.tile([128, 128], FP16)

            def load_x(c, h):
                nc.sync.dma_start(
                    out=xT[:, c, 1024 * h : 1024 * (h + 1)],
                    in_=xT_d[128 * c : 128 * (c + 1), 1024 * h : 1024 * (h + 1)],
                )

            # mask/ident first (tiny, enables PE warmup), then weights, then the
            # s<1024 half of x (everything windows 0-1 need), then the rest:
            # compute starts ~8us in instead of ~20us
            nc.sync.dma_start(
                out=maskD, in_=mask_d.rearrange("p (t c) -> p t c", t=2)
            )
            nc.sync.dma_start(out=ident, in_=id_d[:, :])
            nc.sync.dma_start(out=wq, in_=wq_d.rearrange("(c p) d -> p c d", p=128))
            nc.sync.dma_start(out=wk, in_=wk_d.rearrange("(c p) d -> p c d", p=128))
            for c in range(EC):
                load_x(c, 0)
            nc.sync.dma_start(out=wv, in_=wv_d.rearrange("(c p) d -> p c d", p=128))
            nc.sync.dma_start(out=wo, in_=wo_d.rearrange("(c p) d -> p c d", p=128))
            for c in range(EC):
                load_x(c, 1)

            QT = work.tile([128, 2, S], FP16)
            KT = work.tile([128, 2, S], FP16)
            V = work.tile([128, NJT, HPC, 65], FP16)
            nc.vector.memset(V[:, :, :, 64:65], 1.0)
            outTn = work.tile([128, 2, S], FP16)  # [d-of-pair, hp, s], normalized

            def proj_qk(w_t, dst, hp, st, warm=False):
                ps = psBig.tile([128, 2, 512], FP32, tag="big")
                for c in range(EC):
                    nc.tensor.matmul(
                        ps[:, 0, :],
                        w_t[:, c, 128 * hp : 128 * (hp + 1)],
                        xT[:, c, 512 * st : 512 * (st + 1)],
                        start=(c == 0),
                        stop=(c == EC - 1),
                    )
                    if warm:
                        # x arrives by DMA slower than PE consumes it here;
                        # dummy transposes keep the HAM activity monitor from
                        # throttling the PE clock during the load
                        dt = psSm.tile([128, 128], FP16, tag="sm")
                        for _ in range(6):
                            nc.tensor.transpose(dt, ident, ident)
                nc.vector.tensor_copy(
                    out=dst[:, hp, 512 * st : 512 * (st + 1)], in_=ps[:, 0, :]
                )

            def proj_v(jt):
                ps = psBig.tile([128, 2, 512], FP32, tag="big")
                psd = ps[:, 0, 0:DC]
                for c in range(EC):
                    nc.tensor.matmul(
                        psd,
                        xT[:, c, 128 * jt : 128 * (jt + 1)],
                        wv[:, c, :],
                        start=(c == 0),
                        stop=(c == EC - 1),
                    )
                nc.vector.tensor_copy(
                    out=V[:, jt, :, 0:64],
                    in_=psd.rearrange("p (h d) -> p h d", h=HPC),
                )

            # deferred PE transposes: (hp, sb, os_tile)
            pending_t = []

            def emit_transpose(use_act=False):
                hp, sb, os_t = pending_t.pop(0)
                pt = psSm.tile([128, 128], FP16, tag="sm")
                nc.tensor.transpose(pt, os_t, ident)
                # copyback engine: ACT only at the tail (it is exp-bound
                # during attention; idle once scores are done)
                dst = outTn[:, hp, 128 * sb : 128 * (sb + 1)]
                if use_act:
                    nc.scalar.copy(out=dst, in_=pt)
                else:
                    nc.vector.tensor_copy(out=dst, in_=pt)

            def emit_final(sb):
                tail = sb >= 12
                # transposes this final depends on must be emitted first
                while pending_t and (
                    pending_t[0][0] == 0 or pending_t[0][1] <= sb
                ):
                    emit_transpose(use_act=tail)
                psf = psBig.tile([128, 2, 512], FP32, tag="big")
                for mt in range(2):
                    for hp in range(2):
                        nc.tensor.matmul(
                            psf[:, mt, :],
                            outTn[:, hp, 128 * sb : 128 * (sb + 1)],
                            wo[:, hp, 512 * mt : 512 * (mt + 1)],
                            start=(hp == 0),
                            stop=(hp == 1),
                        )
                ob = obpool.tile([128, 2, 512], FP16, tag="ob")
                if tail:
                    nc.scalar.copy(out=ob, in_=psf)
                else:
                    nc.vector.tensor_copy(out=ob, in_=psf)
                nc.sync.dma_start(
                    out=out_d[128 * sb : 128 * (sb + 1), :].rearrange(
                        "p (t c) -> p t c", t=2
                    ),
                    in_=ob,
                )

            def av_chain_thunks(hp, st, Ut):
                """Two thunks per s-block (one per head) for fine weave pacing;
                the second does normalize + pending push."""
                pos = {}

                def chain(sbl, hi):
                    sb = 4 * st + sbl
                    if hi == 0:
                        pos[sbl] = psSm.tile([128, 2, 65], FP32, tag="sm", name="po")
                    po = pos[sbl]
                    for k in range(sb + 1):
                        nc.tensor.matmul(
                            po[:, hi, :],
                            Ut[:, hi, k, 128 * sbl : 128 * (sbl + 1)],
                            V[:, k, 2 * hp + hi, :],
                            start=(k == 0),
                            stop=(k == sb),
                        )
                    if hi == 0:
                        return
                    zr = zpool.tile([128, 2, 1], FP32, tag="zr")
                    nc.vector.reciprocal(out=zr, in_=po[:, :, 64:65])
                    os_t = ospool.tile([128, 2, DK], FP16, tag="os")
                    for h2 in range(2):
                        nc.vector.tensor_scalar_mul(
                            os_t[:, h2, :], po[:, h2, 0:64], zr[:, h2, :]
                        )
                    pending_t.append((hp, sb, os_t))
                    thresh = 2 if (hp == 1 and st == 3) else 4
                    while len(pending_t) > thresh:
                        emit_transpose()

                return [
                    lambda sbl=sbl, hi=hi: chain(sbl, hi)
                    for sbl in range(4)
                    for hi in range(2)
                ]

            def attn_scores(hp, st, work_items):
                """Emit scores+exp for window st, weaving work_items (AV chains
                of the previous window, projections, finals) between score
                matmuls to keep PE dense. Returns this window's AV thunks."""
                Ut = upool.tile([128, 2, NJT, 512], FP16, tag="U")
                njt = 4 * st + 4
                emitted = 0
                for jt in range(njt):
                    off = max(0, 128 * jt - 512 * st)
                    n = 512 - off
                    ps = psBig.tile([128, 2, 512], FP32, tag="big")
                    for hi in range(2):
                        ho = 64 * hi
                        nc.tensor.matmul(
                            ps[:, hi, 0:n],
                            KT[ho : ho + 64, hp, 128 * jt : 128 * (jt + 1)],
                            QT[ho : ho + 64, hp, 512 * st + off : 512 * (st + 1)],
                            start=True,
                            stop=True,
                        )
                    nc.scalar.activation(
                        out=Ut[:, :, jt, off : off + n],
                        in_=ps[:, :, 0:n],
                        func=mybir.ActivationFunctionType.Exp,
                        scale=0.125,
                    )
                    if jt >= 4 * st:  # diagonal 128-block: causal mask
                        nc.vector.tensor_mul(
                            Ut[:, :, jt, off : off + 128],
                            Ut[:, :, jt, off : off + 128],
                            maskD,
                        )
                    # spread work_items evenly across the jt loop
                    want = (len(work_items) + emitted) * (jt + 1) // njt
                    while emitted < want and work_items:
                        work_items.pop(0)()
                        emitted += 1
                while work_items:
                    work_items.pop(0)()
                return av_chain_thunks(hp, st, Ut)

            # ---- phase A: just window 0's projections; the rest is woven ----
            proj_qk(wq, QT, 0, 0)
            proj_qk(wk, KT, 0, 0)
            for jt in range(4):
                proj_v(jt)
            proj_qk(wq, QT, 1, 0)
            proj_qk(wk, KT, 1, 0)

            # ---- attention: hp=0 then hp=1, with cross-window weaving ----
            av_prev = []
            for st in range(NST):
                fills = []
                if st + 1 < NST:
                    nst = st + 1
                    fills = (
                        [
                            lambda nst=nst: proj_qk(wq, QT, 0, nst),
                            lambda nst=nst: proj_qk(wk, KT, 0, nst),
                        ]
                        + [lambda jt=jt: proj_v(jt) for jt in range(4 * nst, 4 * nst + 4)]
                        + [
                            lambda nst=nst: proj_qk(wq, QT, 1, nst),
                            lambda nst=nst: proj_qk(wk, KT, 1, nst),
                        ]
                    )
                av_prev = attn_scores(0, st, av_prev + fills)
            for st in range(NST):
                fills = []
                if st == 2:
                    fills = [lambda sb=sb: emit_final(sb) for sb in range(0, 4)]
                elif st == 3:
                    fills = [lambda sb=sb: emit_final(sb) for sb in range(4, 12)]
                av_prev = attn_scores(1, st, av_prev + fills)
            # tail: last window's AV chains interleaved with remaining finals
            # (a final may only follow the hi=1 thunk that pushes its transpose)
            finals = list(range(12, 16))
            for i, t in enumerate(av_prev):
                t()
                if i % 2 == 1 and finals:
                    emit_final(finals.pop(0))
            while finals:
                emit_final(finals.pop(0))

    nc.compile()
    return nc


_NC = None


def _prep_in_maps(x, W_q, W_k, W_v, W_o):
    x = np.asarray(x, dtype=np.float32)
    W_q = np.asarray(W_q, dtype=np.float32)
    W_k = np.asarray(W_k, dtype=np.float32)
    W_v = np.asarray(W_v, dtype=np.float32)
    W_o = np.asarray(W_o, dtype=np.float32)
    mask01 = np.triu(np.ones((128, 128), dtype=np.float16))
    mask2 = np.concatenate([mask01, mask01], axis=1)
    ident = np.eye(128, dtype=np.float16)
    in_maps = []
    for c in range(NCORES):
        b, g = divmod(c, 4)
        cols = slice(DC * g, DC * (g + 1))
        in_maps.append(
            {
                "xT": np.ascontiguousarray(x[b].T).astype(np.float16),
                "wqT": np.ascontiguousarray(W_q[cols, :].T).astype(np.float16),
                "wkT": np.ascontiguousarray(W_k[cols, :].T).astype(np.float16),
                "wvT": np.ascontiguousarray(W_v[cols, :].T).astype(np.float16),
                "woT": np.ascontiguousarray(W_o[:, cols].T).astype(np.float16),
                "mask": mask2,
                "ident": ident,
            }
        )
    return in_maps


def _run(x, W_q, W_k, W_v, W_o, **spmd_kwargs):
    global _NC
    if _NC is None:
        _NC = _build()
    in_maps = _prep_in_maps(x, W_q, W_k, W_v, W_o)
    res = bass_utils.run_bass_kernel_spmd(
        _NC, in_maps, core_ids=list(range(NCORES)), **spmd_kwargs
    )
    out = np.empty((2, S, D), dtype=np.float32)
    for b in range(2):
        out[b] = (
            res.results[4 * b]["out"].astype(np.float32)
            + res.results[4 * b + 1]["out"].astype(np.float32)
            + res.results[4 * b + 2]["out"].astype(np.float32)
            + res.results[4 * b + 3]["out"].astype(np.float32)
        )
    return out, res


def kernel(x, W_q, W_k, W_v, W_o):
    out, _ = _run(x, W_q, W_k, W_v, W_o)
    return out


# revision 9
# speedup vs baseline: 1.0066x; 1.0066x over previous
"""Trainium2 Bass kernel v5 for 16-head causal self-attention (KaplanAttention).

Sharding (8 cores): core c handles batch b = c // 4 and head group g = c % 4
(heads 4g..4g+3); host sums the 4 partial output projections per batch.

v5 scheduling insight: exp on ScalarE (~72us) is 2x slower than the warm-clock
score matmuls it consumes (~29us), while ALL other PE work (projections, AV,
transposes, final projection, ~72us) almost exactly matches exp time. So the
emission interleaves at ~0.5us granularity: between every score matmul of
window st, weave an AV chain of window st-1 or a projection/final group.
PE then stays near-100% dense through the whole kernel, which also keeps the
HAM clock gate at 2.4 GHz (sparse PE re-throttles to 1.2 GHz within ~3.4us).

Other structure (from v2-v4): s-major streaming attention; AV flipped so the
softmax denominator lands per-partition (cheap reciprocal + tensor_scalar);
causal mask as a {0,1} DVE multiply on diagonal blocks; deferred PE
transposes bring output back to [d, s] for the final projection; weights
loaded before x; fp16 partial outputs summed on host in fp32.
"""

import numpy as np

from concourse import bass_utils, mybir, tile
from concourse import bacc

S = 2048
D = 1024
HPC = 4        # heads per core
DK = 64
DC = HPC * DK  # 256 d-columns per core
NCORES = 8
EC = D // 128  # 8 e-chunks
NJT = S // 128  # 16 j-tiles
NST = S // 512  # 4 s-windows of 512

FP16 = mybir.dt.float16
FP32 = mybir.dt.float32


def _build():
    nc = bacc.Bacc("TRN2", target_bir_lowering=False, debug=False)

    xT_d = nc.dram_tensor("xT", [D, S], FP16, kind="ExternalInput")
    wq_d = nc.dram_tensor("wqT", [D, DC], FP16, kind="ExternalInput")
    wk_d = nc.dram_tensor("wkT", [D, DC], FP16, kind="ExternalInput")
    wv_d = nc.dram_tensor("wvT", [D, DC], FP16, kind="ExternalInput")
    wo_d = nc.dram_tensor("woT", [DC, D], FP16, kind="ExternalInput")
    mask_d = nc.dram_tensor("mask", [128, 2 * 128], FP16, kind="ExternalInput")
    id_d = nc.dram_tensor("ident", [128, 128], FP16, kind="ExternalInput")
    out_d = nc.dram_tensor("out", [S, D], FP16, kind="ExternalOutput")

    with tile.TileContext(nc) as tc:
        with (
            tc.tile_pool(name="const", bufs=1) as const,
            tc.tile_pool(name="work", bufs=1) as work,
            tc.tile_pool(name="upool", bufs=2) as upool,
            tc.tile_pool(name="ospool", bufs=10) as ospool,
            tc.tile_pool(name="zpool", bufs=8) as zpool,
            tc.tile_pool(name="obpool", bufs=3) as obpool,
            tc.tile_pool(name="psBig", bufs=2, space="PSUM") as psBig,
            tc.tile_pool(name="psPo", bufs=3, space="PSUM") as psPo,
            tc.tile_pool(name="psPt", bufs=1, space="PSUM") as psPt,
        ):
            # ---- load inputs (weights early so compute starts right away) ----
            xT = const.tile([128, EC, S], FP16)
            wq = const.tile([128, EC, DC], FP16)
            wk = const.tile([128, EC, DC], FP16)
            wv = const.tile([128, EC, DC], FP16)
            wo = const.tile([128, 2, D], FP16)
            maskD = const.tile([128, 2, 128], FP16)
            ident = const.tile([128, 128], FP16)

            def load_x(c, h):
                nc.sync.dma_start(
                    out=xT[:, c, 1024 * h : 1024 * (h + 1)],
                    in_=xT_d[128 * c : 128 * (c + 1), 1024 * h : 1024 * (h + 1)],
                )

            # weights, then the s<1024 half of x (everything windows 0-1 need),
            # then the rest: compute starts ~8us in instead of ~20us
            nc.sync.dma_start(out=wq, in_=wq_d.rearrange("(c p) d -> p c d", p=128))
            nc.sync.dma_start(out=wk, in_=wk_d.rearrange("(c p) d -> p c d", p=128))
            for c in range(EC):
                load_x(c, 0)
            nc.sync.dma_start(out=wv, in_=wv_d.rearrange("(c p) d -> p c d", p=128))
            nc.sync.dma_start(out=wo, in_=wo_d.rearrange("(c p) d -> p c d", p=128))
            nc.sync.dma_start(
                out=maskD, in_=mask_d.rearrange("p (t c) -> p t c", t=2)
            )
            nc.sync.dma_start(out=ident, in_=id_d[:, :])
            for c in range(EC):
                load_x(c, 1)

            QT = work.tile([128, 2, S], FP16)
            KT = work.tile([128, 2, S], FP16)
            V = work.tile([128, NJT, HPC, 65], FP16)
            nc.vector.memset(V[:, :, :, 64:65], 1.0)
            outTn = work.tile([128, 2, S], FP16)  # [d-of-pair, hp, s], normalized

            def proj_qk(w_t, dst, hp, st):
                ps = psBig.tile([128, 2, 512], FP32, tag="big")
                for c in range(EC):
                    nc.tensor.matmul(
                        ps[:, 0, :],
                        w_t[:, c, 128 * hp : 128 * (hp + 1)],
                        xT[:, c, 512 * st : 512 * (st + 1)],
                        start=(c == 0),
                        stop=(c == EC - 1),
                    )
                nc.vector.tensor_copy(
                    out=dst[:, hp, 512 * st : 512 * (st + 1)], in_=ps[:, 0, :]
                )

            def proj_v(jt):
                ps = psBig.tile([128, 2, 512], FP32, tag="big")
                psd = ps[:, 0, 0:DC]
                for c in range(EC):
                    nc.tensor.matmul(
                        psd,
                        xT[:, c, 128 * jt : 128 * (jt + 1)],
                        wv[:, c, :],
                        start=(c == 0),
                        stop=(c == EC - 1),
                    )
                nc.vector.tensor_copy(
                    out=V[:, jt, :, 0:64],
                    in_=psd.rearrange("p (h d) -> p h d", h=HPC),
                )

            # deferred PE transposes: (hp, sb, os_tile)
            pending_t = []

            def emit_transpose():
                hp, sb, os_t = pending_t.pop(0)
                pt = psPt.tile([128, 128], FP16, tag="pt")
                nc.tensor.transpose(pt, os_t, ident)
                nc.vector.tensor_copy(
                    out=outTn[:, hp, 128 * sb : 128 * (sb + 1)], in_=pt
                )

            def emit_final(sb):
                # transposes this final depends on must be emitted first
                while pending_t and (
                    pending_t[0][0] == 0 or pending_t[0][1] <= sb
                ):
                    emit_transpose()
                psf = psBig.tile([128, 2, 512], FP32, tag="big")
                for mt in range(2):
                    for hp in range(2):
                        nc.tensor.matmul(
                            psf[:, mt, :],
                            outTn[:, hp, 128 * sb : 128 * (sb + 1)],
                            wo[:, hp, 512 * mt : 512 * (mt + 1)],
                            start=(hp == 0),
                            stop=(hp == 1),
                        )
                ob = obpool.tile([128, 2, 512], FP16, tag="ob")
                if sb % 2 == 0:
                    nc.vector.tensor_copy(out=ob, in_=psf)
                else:
                    nc.scalar.copy(out=ob, in_=psf)
                nc.sync.dma_start(
                    out=out_d[128 * sb : 128 * (sb + 1), :].rearrange(
                        "p (t c) -> p t c", t=2
                    ),
                    in_=ob,
                )

            def av_chain_thunks(hp, st, Ut):
                """One thunk per s-block: AV chain + normalize + pending push."""

                def chain(sbl):
                    sb = 4 * st + sbl
                    po = psPo.tile([128, 2, 65], FP32, tag="po")
                    for hi in range(2):
                        for k in range(sb + 1):
                            nc.tensor.matmul(
                                po[:, hi, :],
                                Ut[:, hi, k, 128 * sbl : 128 * (sbl + 1)],
                                V[:, k, 2 * hp + hi, :],
                                start=(k == 0),
                                stop=(k == sb),
                            )
                    zr = zpool.tile([128, 2, 1], FP32, tag="zr")
                    nc.vector.reciprocal(out=zr, in_=po[:, :, 64:65])
                    os_t = ospool.tile([128, 2, DK], FP16, tag="os")
                    for hi in range(2):
                        nc.vector.tensor_scalar_mul(
                            os_t[:, hi, :], po[:, hi, 0:64], zr[:, hi, :]
                        )
                    pending_t.append((hp, sb, os_t))
                    if len(pending_t) > 4:
                        emit_transpose()

                return [lambda sbl=sbl: chain(sbl) for sbl in range(4)]

            def attn_scores(hp, st, work_items):
                """Emit scores+exp for window st, weaving work_items (AV chains
                of the previous window, projections, finals) between score
                matmuls to keep PE dense. Returns this window's AV thunks."""
                Ut = upool.tile([128, 2, NJT, 512], FP16, tag="U")
                njt = 4 * st + 4
                emitted = 0
                for jt in range(njt):
                    off = max(0, 128 * jt - 512 * st)
                    n = 512 - off
                    ps = psBig.tile([128, 2, 512], FP32, tag="big")
                    for hi in range(2):
                        ho = 64 * hi
                        nc.tensor.matmul(
                            ps[:, hi, 0:n],
                            KT[ho : ho + 64, hp, 128 * jt : 128 * (jt + 1)],
                            QT[ho : ho + 64, hp, 512 * st + off : 512 * (st + 1)],
                            start=True,
                            stop=True,
                        )
                    nc.scalar.activation(
                        out=Ut[:, :, jt, off : off + n],
                        in_=ps[:, :, 0:n],
                        func=mybir.ActivationFunctionType.Exp,
                        scale=0.125,
                    )
                    if jt >= 4 * st:  # diagonal 128-block: causal mask
                        nc.vector.tensor_mul(
                            Ut[:, :, jt, off : off + 128],
                            Ut[:, :, jt, off : off + 128],
                            maskD,
                        )
                    # spread work_items evenly across the jt loop
                    want = (len(work_items) + emitted) * (jt + 1) // njt
                    while emitted < want and work_items:
                        work_items.pop(0)()
                        emitted += 1
                while work_items:
                    work_items.pop(0)()
                return av_chain_thunks(hp, st, Ut)

            # ---- phase A: just window 0's projections; the rest is woven ----
            proj_qk(wq, QT, 0, 0)
            proj_qk(wk, KT, 0, 0)
            for jt in range(4):
                proj_v(jt)
            proj_qk(wq, QT, 1, 0)
            proj_qk(wk, KT, 1, 0)

            # ---- attention: hp=0 then hp=1, with cross-window weaving ----
            av_prev = []
            for st in range(NST):
                fills = []
                if st + 1 < NST:
                    nst = st + 1
                    fills = (
                        [
                            lambda nst=nst: proj_qk(wq, QT, 0, nst),
                            lambda nst=nst: proj_qk(wk, KT, 0, nst),
                        ]
                        + [lambda jt=jt: proj_v(jt) for jt in range(4 * nst, 4 * nst + 4)]
                        + [
                            lambda nst=nst: proj_qk(wq, QT, 1, nst),
                            lambda nst=nst: proj_qk(wk, KT, 1, nst),
                        ]
                    )
                av_prev = attn_scores(0, st, av_prev + fills)
            for st in range(NST):
                fills = []
                if st >= 1:
                    fills = [
                        lambda sb=sb: emit_final(sb)
                        for sb in range(4 * (st - 1), 4 * st)
                    ]
                av_prev = attn_scores(1, st, av_prev + fills)
            # tail: last window's AV chains interleaved with remaining finals
            finals = list(range(12, 16))
            for t in av_prev:
                t()
                if finals:
                    emit_final(finals.pop(0))
            while finals:
                emit_final(finals.pop(0))

    nc.compile()
    return nc


_NC = None


def _prep_in_maps(x, W_q, W_k, W_v, W_o):
    x = np.asarray(x, dtype=np.float32)
    W_q = np.asarray(W_q, dtype=np.float32)
    W_k = np.asarray(W_k, dtype=np.float32)
    W_v = np.asarray(W_v, dtype=np.float32)
    W_o = np.asarray(W_o, dtype=np.float32)
    mask01 = np.triu(np.ones((128, 128), dtype=np.float16))
    mask2 = np.concatenate([mask01, mask01], axis=1)
    ident = np.eye(128, dtype=np.float16)
    in_maps = []
    for c in range(NCORES):
        b, g = divmod(c, 4)
        cols = slice(DC * g, DC * (g + 1))
        in_maps.append(
            {
                "xT": np.ascontiguousarray(x[b].T).astype(np.float16),
                "wqT": np.ascontiguousarray(W_q[cols, :].T).astype(np.float16),
                "wkT": np.ascontiguousarray(W_k[cols, :].T).astype(np.float16),
                "wvT": np.ascontiguousarray(W_v[cols, :].T).astype(np.float16),
                "woT": np.ascontiguousarray(W_o[:, cols].T).astype(np.float16),
                "mask": mask2,
                "ident": ident,
            }
        )
    return in_maps


def _run(x, W_q, W_k, W_v, W_o, **spmd_kwargs):
    global _NC
    if _NC is None:
        _NC = _build()
    in_maps = _prep_in_maps(x, W_q, W_k, W_v, W_o)
    res = bass_utils.run_bass_kernel_spmd(
        _NC, in_maps, core_ids=list(range(NCORES)), **spmd_kwargs
    )
    out = np.empty((2, S, D), dtype=np.float32)
    for b in range(2):
        out[b] = (
            res.results[4 * b]["out"].astype(np.float32)
            + res.results[4 * b + 1]["out"].astype(np.float32)
            + res.results[4 * b + 2]["out"].astype(np.float32)
            + res.results[4 * b + 3]["out"].astype(np.float32)
        )
    return out, res


def kernel(x, W_q, W_k, W_v, W_o):
    out, _ = _run(x, W_q, W_k, W_v, W_o)
    return out


# revision 10
# speedup vs baseline: 1.0230x; 1.0164x over previous
"""Trainium2 Bass kernel v5 for 16-head causal self-attention (KaplanAttention).

Sharding (8 cores): core c handles batch b = c // 4 and head group g = c % 4
(heads 4g..4g+3); host sums the 4 partial output projections per batch.

v5 scheduling insight: exp on ScalarE (~72us) is 2x slower than the warm-clock
score matmuls it consumes (~29us), while ALL other PE work (projections, AV,
transposes, final projection, ~72us) almost exactly matches exp time. So the
emission interleaves at ~0.5us granularity: between every score matmul of
window st, weave an AV chain of window st-1 or a projection/final group.
PE then stays near-100% dense through the whole kernel, which also keeps the
HAM clock gate at 2.4 GHz (sparse PE re-throttles to 1.2 GHz within ~3.4us).

Other structure (from v2-v4): s-major streaming attention; AV flipped so the
softmax denominator lands per-partition (cheap reciprocal + tensor_scalar);
causal mask as a {0,1} DVE multiply on diagonal blocks; deferred PE
transposes bring output back to [d, s] for the final projection; weights
loaded before x; fp16 partial outputs summed on host in fp32.
"""

import numpy as np

from concourse import bass_utils, mybir, tile
from concourse import bacc

S = 2048
D = 1024
HPC = 4        # heads per core
DK = 64
DC = HPC * DK  # 256 d-columns per core
NCORES = 8
EC = D // 128  # 8 e-chunks
NJT = S // 128  # 16 j-tiles
NST = S // 512  # 4 s-windows of 512

FP16 = mybir.dt.float16
FP32 = mybir.dt.float32


def _build():
    nc = bacc.Bacc("TRN2", target_bir_lowering=False, debug=False)

    xT_d = nc.dram_tensor("xT", [D, S], FP16, kind="ExternalInput")
    wq_d = nc.dram_tensor("wqT", [D, DC], FP16, kind="ExternalInput")
    wk_d = nc.dram_tensor("wkT", [D, DC], FP16, kind="ExternalInput")
    wv_d = nc.dram_tensor("wvT", [D, DC], FP16, kind="ExternalInput")
    wo_d = nc.dram_tensor("woT", [DC, D], FP16, kind="ExternalInput")
    mask_d = nc.dram_tensor("mask", [128, 2 * 128], FP16, kind="ExternalInput")
    id_d = nc.dram_tensor("ident", [128, 128], FP16, kind="ExternalInput")
    out_d = nc.dram_tensor("out", [S, D], FP16, kind="ExternalOutput")

    with tile.TileContext(nc) as tc:
        with (
            tc.tile_pool(name="const", bufs=1) as const,
            tc.tile_pool(name="work", bufs=1) as work,
            tc.tile_pool(name="upool", bufs=2) as upool,
            tc.tile_pool(name="ospool", bufs=10) as ospool,
            tc.tile_pool(name="zpool", bufs=8) as zpool,
            tc.tile_pool(name="obpool", bufs=3) as obpool,
            tc.tile_pool(name="psBig", bufs=2, space="PSUM") as psBig,
            tc.tile_pool(name="psPo", bufs=3, space="PSUM") as psPo,
            tc.tile_pool(name="psPt", bufs=1, space="PSUM") as psPt,
        ):
            # ---- load inputs (weights early so compute starts right away) ----
            xT = const.tile([128, EC, S], FP16)
            wq = const.tile([128, EC, DC], FP16)
            wk = const.tile([128, EC, DC], FP16)
            wv = const.tile([128, EC, DC], FP16)
            wo = const.tile([128, 2, D], FP16)
            maskD = const.tile([128, 2, 128], FP16)
            ident = const.tile([128, 128], FP16)

            def load_x(c, h):
                nc.sync.dma_start(
                    out=xT[:, c, 1024 * h : 1024 * (h + 1)],
                    in_=xT_d[128 * c : 128 * (c + 1), 1024 * h : 1024 * (h + 1)],
                )

            # weights, then the s<1024 half of x (everything windows 0-1 need),
            # then the rest: compute starts ~8us in instead of ~20us
            nc.sync.dma_start(out=wq, in_=wq_d.rearrange("(c p) d -> p c d", p=128))
            nc.sync.dma_start(out=wk, in_=wk_d.rearrange("(c p) d -> p c d", p=128))
            for c in range(EC):
                load_x(c, 0)
            nc.sync.dma_start(out=wv, in_=wv_d.rearrange("(c p) d -> p c d", p=128))
            nc.sync.dma_start(out=wo, in_=wo_d.rearrange("(c p) d -> p c d", p=128))
            nc.sync.dma_start(
                out=maskD, in_=mask_d.rearrange("p (t c) -> p t c", t=2)
            )
            nc.sync.dma_start(out=ident, in_=id_d[:, :])
            for c in range(EC):
                load_x(c, 1)

            QT = work.tile([128, 2, S], FP16)
            KT = work.tile([128, 2, S], FP16)
            V = work.tile([128, NJT, HPC, 65], FP16)
            nc.vector.memset(V[:, :, :, 64:65], 1.0)
            outTn = work.tile([128, 2, S], FP16)  # [d-of-pair, hp, s], normalized

            def proj_qk(w_t, dst, hp, st):
                ps = psBig.tile([128, 2, 512], FP32, tag="big")
                for c in range(EC):
                    nc.tensor.matmul(
                        ps[:, 0, :],
                        w_t[:, c, 128 * hp : 128 * (hp + 1)],
                        xT[:, c, 512 * st : 512 * (st + 1)],
                        start=(c == 0),
                        stop=(c == EC - 1),
                    )
                nc.vector.tensor_copy(
                    out=dst[:, hp, 512 * st : 512 * (st + 1)], in_=ps[:, 0, :]
                )

            def proj_v(jt):
                ps = psBig.tile([128, 2, 512], FP32, tag="big")
                psd = ps[:, 0, 0:DC]
                for c in range(EC):
                    nc.tensor.matmul(
                        psd,
                        xT[:, c, 128 * jt : 128 * (jt + 1)],
                        wv[:, c, :],
                        start=(c == 0),
                        stop=(c == EC - 1),
                    )
                nc.vector.tensor_copy(
                    out=V[:, jt, :, 0:64],
                    in_=psd.rearrange("p (h d) -> p h d", h=HPC),
                )

            # deferred PE transposes: (hp, sb, os_tile)
            pending_t = []

            def emit_transpose():
                hp, sb, os_t = pending_t.pop(0)
                pt = psPt.tile([128, 128], FP16, tag="pt")
                nc.tensor.transpose(pt, os_t, ident)
                nc.vector.tensor_copy(
                    out=outTn[:, hp, 128 * sb : 128 * (sb + 1)], in_=pt
                )

            def emit_final(sb):
                # transposes this final depends on must be emitted first
                while pending_t and (
                    pending_t[0][0] == 0 or pending_t[0][1] <= sb
                ):
                    emit_transpose()
                psf = psBig.tile([128, 2, 512], FP32, tag="big")
                for mt in range(2):
                    for hp in range(2):
                        nc.tensor.matmul(
                            psf[:, mt, :],
                            outTn[:, hp, 128 * sb : 128 * (sb + 1)],
                            wo[:, hp, 512 * mt : 512 * (mt + 1)],
                            start=(hp == 0),
                            stop=(hp == 1),
                        )
                ob = obpool.tile([128, 2, 512], FP16, tag="ob")
                if sb % 2 == 0:
                    nc.vector.tensor_copy(out=ob, in_=psf)
                else:
                    nc.scalar.copy(out=ob, in_=psf)
                nc.sync.dma_start(
                    out=out_d[128 * sb : 128 * (sb + 1), :].rearrange(
                        "p (t c) -> p t c", t=2
                    ),
                    in_=ob,
                )

            def av_chain_thunks(hp, st, Ut):
                """One thunk per s-block: AV chain + normalize + pending push."""

                def chain(sbl):
                    sb = 4 * st + sbl
                    po = psPo.tile([128, 2, 65], FP32, tag="po")
                    for hi in range(2):
                        for k in range(sb + 1):
                            nc.tensor.matmul(
                                po[:, hi, :],
                                Ut[:, hi, k, 128 * sbl : 128 * (sbl + 1)],
                                V[:, k, 2 * hp + hi, :],
                                start=(k == 0),
                                stop=(k == sb),
                            )
                    zr = zpool.tile([128, 2, 1], FP32, tag="zr")
                    nc.vector.reciprocal(out=zr, in_=po[:, :, 64:65])
                    os_t = ospool.tile([128, 2, DK], FP16, tag="os")
                    for hi in range(2):
                        nc.vector.tensor_scalar_mul(
                            os_t[:, hi, :], po[:, hi, 0:64], zr[:, hi, :]
                        )
                    pending_t.append((hp, sb, os_t))
                    if len(pending_t) > 4:
                        emit_transpose()

                return [lambda sbl=sbl: chain(sbl) for sbl in range(4)]

            def attn_scores(hp, st, work_items):
                """Emit scores+exp for window st, weaving work_items (AV chains
                of the previous window, projections, finals) between score
                matmuls to keep PE dense. Returns this window's AV thunks."""
                Ut = upool.tile([128, 2, NJT, 512], FP16, tag="U")
                njt = 4 * st + 4
                emitted = 0
                for jt in range(njt):
                    off = max(0, 128 * jt - 512 * st)
                    n = 512 - off
                    ps = psBig.tile([128, 2, 512], FP32, tag="big")
                    for hi in range(2):
                        ho = 64 * hi
                        nc.tensor.matmul(
                            ps[:, hi, 0:n],
                            KT[ho : ho + 64, hp, 128 * jt : 128 * (jt + 1)],
                            QT[ho : ho + 64, hp, 512 * st + off : 512 * (st + 1)],
                            start=True,
                            stop=True,
                        )
                    nc.scalar.activation(
                        out=Ut[:, :, jt, off : off + n],
                        in_=ps[:, :, 0:n],
                        func=mybir.ActivationFunctionType.Exp,
                        scale=0.125,
                    )
                    if jt >= 4 * st:  # diagonal 128-block: causal mask
                        nc.vector.tensor_mul(
                            Ut[:, :, jt, off : off + 128],
                            Ut[:, :, jt, off : off + 128],
                            maskD,
                        )
                    # spread work_items evenly across the jt loop
                    want = (len(work_items) + emitted) * (jt + 1) // njt
                    while emitted < want and work_items:
                        work_items.pop(0)()
                        emitted += 1
                while work_items:
                    work_items.pop(0)()
                return av_chain_thunks(hp, st, Ut)

            # ---- phase A: just window 0's projections; the rest is woven ----
            proj_qk(wq, QT, 0, 0)
            proj_qk(wk, KT, 0, 0)
            for jt in range(4):
                proj_v(jt)
            proj_qk(wq, QT, 1, 0)
            proj_qk(wk, KT, 1, 0)

            # ---- attention: hp=0 then hp=1, with cross-window weaving ----
            av_prev = []
            for st in range(NST):
                fills = []
                if st + 1 < NST:
                    nst = st + 1
                    fills = (
                        [
                            lambda nst=nst: proj_qk(wq, QT, 0, nst),
                            lambda nst=nst: proj_qk(wk, KT, 0, nst),
                        ]
                        + [lambda jt=jt: proj_v(jt) for jt in range(4 * nst, 4 * nst + 4)]
                        + [
                            lambda nst=nst: proj_qk(wq, QT, 1, nst),
                            lambda nst=nst: proj_qk(wk, KT, 1, nst),
                        ]
                    )
                av_prev = attn_scores(0, st, av_prev + fills)
            for st in range(NST):
                fills = []
                if st == 2:
                    fills = [lambda sb=sb: emit_final(sb) for sb in range(0, 4)]
                elif st == 3:
                    fills = [lambda sb=sb: emit_final(sb) for sb in range(4, 12)]
                av_prev = attn_scores(1, st, av_prev + fills)
            # tail: last window's AV chains interleaved with remaining finals
            finals = list(range(12, 16))
            for t in av_prev:
                t()
                if finals:
                    emit_final(finals.pop(0))
            while finals:
                emit_final(finals.pop(0))

    nc.compile()
    return nc


_NC = None


def _prep_in_maps(x, W_q, W_k, W_v, W_o):
    x = np.asarray(x, dtype=np.float32)
    W_q = np.asarray(W_q, dtype=np.float32)
    W_k = np.asarray(W_k, dtype=np.float32)
    W_v = np.asarray(W_v, dtype=np.float32)
    W_o = np.asarray(W_o, dtype=np.float32)
    mask01 = np.triu(np.ones((128, 128), dtype=np.float16))
    mask2 = np.concatenate([mask01, mask01], axis=1)
    ident = np.eye(128, dtype=np.float16)
    in_maps = []
    for c in range(NCORES):
        b, g = divmod(c, 4)
        cols = slice(DC * g, DC * (g + 1))
        in_maps.append(
            {
                "xT": np.ascontiguousarray(x[b].T).astype(np.float16),
                "wqT": np.ascontiguousarray(W_q[cols, :].T).astype(np.float16),
                "wkT": np.ascontiguousarray(W_k[cols, :].T).astype(np.float16),
                "wvT": np.ascontiguousarray(W_v[cols, :].T).astype(np.float16),
                "woT": np.ascontiguousarray(W_o[:, cols].T).astype(np.float16),
                "mask": mask2,
                "ident": ident,
            }
        )
    return in_maps


def _run(x, W_q, W_k, W_v, W_o, **spmd_kwargs):
    global _NC
    if _NC is None:
        _NC = _build()
    in_maps = _prep_in_maps(x, W_q, W_k, W_v, W_o)
    res = bass_utils.run_bass_kernel_spmd(
        _NC, in_maps, core_ids=list(range(NCORES)), **spmd_kwargs
    )
    out = np.empty((2, S, D), dtype=np.float32)
    for b in range(2):
        out[b] = (
            res.results[4 * b]["out"].astype(np.float32)
            + res.results[4 * b + 1]["out"].astype(np.float32)
            + res.results[4 * b + 2]["out"].astype(np.float32)
            + res.results[4 * b + 3]["out"].astype(np.float32)
        )
    return out, res


def kernel(x, W_q, W_k, W_v, W_o):
    out, _ = _run(x, W_q, W_k, W_v, W_o)
    return out


# revision 11
# speedup vs baseline: 1.1349x; 1.1093x over previous
"""Trainium2 Bass kernel v5 for 16-head causal self-attention (KaplanAttention).

Sharding (8 cores): core c handles batch b = c // 4 and head group g = c % 4
(heads 4g..4g+3); host sums the 4 partial output projections per batch.

v5 scheduling insight: exp on ScalarE (~72us) is 2x slower than the warm-clock
score matmuls it consumes (~29us), while ALL other PE work (projections, AV,
transposes, final projection, ~72us) almost exactly matches exp time. So the
emission interleaves at ~0.5us granularity: between every score matmul of
window st, weave an AV chain of window st-1 or a projection/final group.
PE then stays near-100% dense through the whole kernel, which also keeps the
HAM clock gate at 2.4 GHz (sparse PE re-throttles to 1.2 GHz within ~3.4us).

Other structure (from v2-v4): s-major streaming attention; AV flipped so the
softmax denominator lands per-partition (cheap reciprocal + tensor_scalar);
causal mask as a {0,1} DVE multiply on diagonal blocks; deferred PE
transposes bring output back to [d, s] for the final projection; weights
loaded before x; fp16 partial outputs summed on host in fp32.
"""

import numpy as np

from concourse import bass_utils, mybir, tile
from concourse import bacc

S = 2048
D = 1024
HPC = 4        # heads per core
DK = 64
DC = HPC * DK  # 256 d-columns per core
NCORES = 8
EC = D // 128  # 8 e-chunks
NJT = S // 128  # 16 j-tiles
NST = S // 512  # 4 s-windows of 512

FP16 = mybir.dt.float16
FP32 = mybir.dt.float32


def _build():
    nc = bacc.Bacc("TRN2", target_bir_lowering=False, debug=False)

    xT_d = nc.dram_tensor("xT", [D, S], FP16, kind="ExternalInput")
    wq_d = nc.dram_tensor("wqT", [D, DC], FP16, kind="ExternalInput")
    wk_d = nc.dram_tensor("wkT", [D, DC], FP16, kind="ExternalInput")
    wv_d = nc.dram_tensor("wvT", [D, DC], FP16, kind="ExternalInput")
    wo_d = nc.dram_tensor("woT", [DC, D], FP16, kind="ExternalInput")
    mask_d = nc.dram_tensor("mask", [128, 2 * 128], FP16, kind="ExternalInput")
    id_d = nc.dram_tensor("ident", [128, 128], FP16, kind="ExternalInput")
    out_d = nc.dram_tensor("out", [S, D], FP16, kind="ExternalOutput")

    with tile.TileContext(nc) as tc:
        with (
            tc.tile_pool(name="const", bufs=1) as const,
            tc.tile_pool(name="work", bufs=1) as work,
            tc.tile_pool(name="upool", bufs=2) as upool,
            tc.tile_pool(name="ospool", bufs=10) as ospool,
            tc.tile_pool(name="zpool", bufs=8) as zpool,
            tc.tile_pool(name="obpool", bufs=3) as obpool,
            tc.tile_pool(name="psBig", bufs=2, space="PSUM") as psBig,
            tc.tile_pool(name="psPo", bufs=3, space="PSUM") as psPo,
            tc.tile_pool(name="psPt", bufs=1, space="PSUM") as psPt,
        ):
            # ---- load inputs (weights early so compute starts right away) ----
            xT = const.tile([128, EC, S], FP16)
            wq = const.tile([128, EC, DC], FP16)
            wk = const.tile([128, EC, DC], FP16)
            wv = const.tile([128, EC, DC], FP16)
            wo = const.tile([128, 2, D], FP16)
            maskD = const.tile([128, 2, 128], FP16)
            ident = const.tile([128, 128], FP16)

            def load_x(c, h):
                nc.sync.dma_start(
                    out=xT[:, c, 1024 * h : 1024 * (h + 1)],
                    in_=xT_d[128 * c : 128 * (c + 1), 1024 * h : 1024 * (h + 1)],
                )

            # weights, then the s<1024 half of x (everything windows 0-1 need),
            # then the rest: compute starts ~8us in instead of ~20us
            nc.sync.dma_start(out=wq, in_=wq_d.rearrange("(c p) d -> p c d", p=128))
            nc.sync.dma_start(out=wk, in_=wk_d.rearrange("(c p) d -> p c d", p=128))
            for c in range(EC):
                load_x(c, 0)
            nc.sync.dma_start(out=wv, in_=wv_d.rearrange("(c p) d -> p c d", p=128))
            nc.sync.dma_start(out=wo, in_=wo_d.rearrange("(c p) d -> p c d", p=128))
            nc.sync.dma_start(
                out=maskD, in_=mask_d.rearrange("p (t c) -> p t c", t=2)
            )
            nc.sync.dma_start(out=ident, in_=id_d[:, :])
            for c in range(EC):
                load_x(c, 1)

            QT = work.tile([128, 2, S], FP16)
            KT = work.tile([128, 2, S], FP16)
            V = work.tile([128, NJT, HPC, 65], FP16)
            nc.vector.memset(V[:, :, :, 64:65], 1.0)
            outTn = work.tile([128, 2, S], FP16)  # [d-of-pair, hp, s], normalized

            def proj_qk(w_t, dst, hp, st):
                ps = psBig.tile([128, 2, 512], FP32, tag="big")
                for c in range(EC):
                    nc.tensor.matmul(
                        ps[:, 0, :],
                        w_t[:, c, 128 * hp : 128 * (hp + 1)],
                        xT[:, c, 512 * st : 512 * (st + 1)],
                        start=(c == 0),
                        stop=(c == EC - 1),
                    )
                nc.vector.tensor_copy(
                    out=dst[:, hp, 512 * st : 512 * (st + 1)], in_=ps[:, 0, :]
                )

            def proj_v(jt):
                ps = psBig.tile([128, 2, 512], FP32, tag="big")
                psd = ps[:, 0, 0:DC]
                for c in range(EC):
                    nc.tensor.matmul(
                        psd,
                        xT[:, c, 128 * jt : 128 * (jt + 1)],
                        wv[:, c, :],
                        start=(c == 0),
                        stop=(c == EC - 1),
                    )
                nc.vector.tensor_copy(
                    out=V[:, jt, :, 0:64],
                    in_=psd.rearrange("p (h d) -> p h d", h=HPC),
                )

            # deferred PE transposes: (hp, sb, os_tile)
            pending_t = []

            def emit_transpose(use_act=False):
                hp, sb, os_t = pending_t.pop(0)
                pt = psPt.tile([128, 128], FP16, tag="pt")
                nc.tensor.transpose(pt, os_t, ident)
                dst = outTn[:, hp, 128 * sb : 128 * (sb + 1)]
                if use_act:
                    nc.scalar.copy(out=dst, in_=pt)
                else:
                    nc.vector.tensor_copy(out=dst, in_=pt)

            def emit_final(sb):
                # transposes this final depends on must be emitted first
                while pending_t and (
                    pending_t[0][0] == 0 or pending_t[0][1] <= sb
                ):
                    emit_transpose(use_act=(sb >= 12))
                psf = psBig.tile([128, 2, 512], FP32, tag="big")
                for mt in range(2):
                    for hp in range(2):
                        nc.tensor.matmul(
                            psf[:, mt, :],
                            outTn[:, hp, 128 * sb : 128 * (sb + 1)],
                            wo[:, hp, 512 * mt : 512 * (mt + 1)],
                            start=(hp == 0),
                            stop=(hp == 1),
                        )
                ob = obpool.tile([128, 2, 512], FP16, tag="ob")
                if sb >= 12:
                    nc.scalar.copy(out=ob, in_=psf)
                else:
                    nc.vector.tensor_copy(out=ob, in_=psf)
                nc.sync.dma_start(
                    out=out_d[128 * sb : 128 * (sb + 1), :].rearrange(
                        "p (t c) -> p t c", t=2
                    ),
                    in_=ob,
                )

            def av_chain_thunks(hp, st, Ut):
                """One thunk per s-block: AV chain + normalize + pending push."""

                def chain(sbl):
                    sb = 4 * st + sbl
                    po = psPo.tile([128, 2, 65], FP32, tag="po")
                    for hi in range(2):
                        for k in range(sb + 1):
                            nc.tensor.matmul(
                                po[:, hi, :],
                                Ut[:, hi, k, 128 * sbl : 128 * (sbl + 1)],
                                V[:, k, 2 * hp + hi, :],
                                start=(k == 0),
                                stop=(k == sb),
                            )
                    zr = zpool.tile([128, 2, 1], FP32, tag="zr")
                    nc.vector.reciprocal(out=zr, in_=po[:, :, 64:65])
                    os_t = ospool.tile([128, 2, DK], FP16, tag="os")
                    for hi in range(2):
                        nc.vector.tensor_scalar_mul(
                            os_t[:, hi, :], po[:, hi, 0:64], zr[:, hi, :]
                        )
                    pending_t.append((hp, sb, os_t))
                    thresh = 2 if (hp == 1 and st == 3) else 4
                    while len(pending_t) > thresh:
                        emit_transpose()

                return [lambda sbl=sbl: chain(sbl) for sbl in range(4)]

            def attn_scores(hp, st, work_items):
                """Emit scores+exp for window st, weaving work_items (AV chains
                of the previous window, projections, finals) between score
                matmuls to keep PE dense. Returns this window's AV thunks."""
                Ut = upool.tile([128, 2, NJT, 512], FP16, tag="U")
                njt = 4 * st + 4
                emitted = 0
                for jt in range(njt):
                    off = max(0, 128 * jt - 512 * st)
                    n = 512 - off
                    ps = psBig.tile([128, 2, 512], FP32, tag="big")
                    for hi in range(2):
                        ho = 64 * hi
                        nc.tensor.matmul(
                            ps[:, hi, 0:n],
                            KT[ho : ho + 64, hp, 128 * jt : 128 * (jt + 1)],
                            QT[ho : ho + 64, hp, 512 * st + off : 512 * (st + 1)],
                            start=True,
                            stop=True,
                        )
                    nc.scalar.activation(
                        out=Ut[:, :, jt, off : off + n],
                        in_=ps[:, :, 0:n],
                        func=mybir.ActivationFunctionType.Exp,
                        scale=0.125,
                    )
                    if jt >= 4 * st:  # diagonal 128-block: causal mask
                        nc.vector.tensor_mul(
                            Ut[:, :, jt, off : off + 128],
                            Ut[:, :, jt, off : off + 128],
                            maskD,
                        )
                    # spread work_items evenly across the jt loop
                    want = (len(work_items) + emitted) * (jt + 1) // njt
                    while emitted < want and work_items:
                        work_items.pop(0)()
                        emitted += 1
                while work_items:
                    work_items.pop(0)()
                return av_chain_thunks(hp, st, Ut)

            # ---- phase A: just window 0's projections; the rest is woven ----
            proj_qk(wq, QT, 0, 0)
            proj_qk(wk, KT, 0, 0)
            for jt in range(4):
                proj_v(jt)
            proj_qk(wq, QT, 1, 0)
            proj_qk(wk, KT, 1, 0)

            # ---- attention: hp=0 then hp=1, with cross-window weaving ----
            av_prev = []
            for st in range(NST):
                fills = []
                if st + 1 < NST:
                    nst = st + 1
                    fills = (
                        [
                            lambda nst=nst: proj_qk(wq, QT, 0, nst),
                            lambda nst=nst: proj_qk(wk, KT, 0, nst),
                        ]
                        + [lambda jt=jt: proj_v(jt) for jt in range(4 * nst, 4 * nst + 4)]
                        + [
                            lambda nst=nst: proj_qk(wq, QT, 1, nst),
                            lambda nst=nst: proj_qk(wk, KT, 1, nst),
                        ]
                    )
                av_prev = attn_scores(0, st, av_prev + fills)
            for st in range(NST):
                fills = []
                if st == 2:
                    fills = [lambda sb=sb: emit_final(sb) for sb in range(0, 4)]
                elif st == 3:
                    fills = [lambda sb=sb: emit_final(sb) for sb in range(4, 12)]
                av_prev = attn_scores(1, st, av_prev + fills)
            # tail: last window's AV chains interleaved with remaining finals
            finals = list(range(12, 16))
            for t in av_prev:
                t()
                if finals:
                    emit_final(finals.pop(0))
            while finals:
                emit_final(finals.pop(0))

    nc.compile()
    return nc


_NC = None


def _prep_in_maps(x, W_q, W_k, W_v, W_o):
    x = np.asarray(x, dtype=np.float32)
    W_q = np.asarray(W_q, dtype=np.float32)
    W_k = np.asarray(W_k, dtype=np.float32)
    W_v = np.asarray(W_v, dtype=np.float32)
    W_o = np.asarray(W_o, dtype=np.float32)
    mask01 = np.triu(np.ones((128, 128), dtype=np.float16))
    mask2 = np.concatenate([mask01, mask01], axis=1)
    ident = np.eye(128, dtype=np.float16)
    in_maps = []
    for c in range(NCORES):
        b, g = divmod(c, 4)
        cols = slice(DC * g, DC * (g + 1))
        in_maps.append(
            {
                "xT": np.ascontiguousarray(x[b].T).astype(np.float16),
                "wqT": np.ascontiguousarray(W_q[cols, :].T).astype(np.float16),
                "wkT": np.ascontiguousarray(W_k[cols, :].T).astype(np.float16),
                "wvT": np.ascontiguousarray(W_v[cols, :].T).astype(np.float16),
                "woT": np.ascontiguousarray(W_o[:, cols].T).astype(np.float16),
                "mask": mask2,
                "ident": ident,
            }
        )
    return in_maps


def _run(x, W_q, W_k, W_v, W_o, **spmd_kwargs):
    global _NC
    if _NC is None:
        _NC = _build()
    in_maps = _prep_in_maps(x, W_q, W_k, W_v, W_o)
    res = bass_utils.run_bass_kernel_spmd(
        _NC, in_maps, core_ids=list(range(NCORES)), **spmd_kwargs
    )
    out = np.empty((2, S, D), dtype=np.float32)
    for b in range(2):
        out[b] = (
            res.results[4 * b]["out"].astype(np.float32)
            + res.results[4 * b + 1]["out"].astype(np.float32)
            + res.results[4 * b + 2]["out"].astype(np.float32)
            + res.results[4 * b + 3]["out"].astype(np.float32)
        )
    return out, res


def kernel(x, W_q, W_k, W_v, W_o):
    out, _ = _run(x, W_q, W_k, W_v, W_o)
    return out


# revision 12
# speedup vs baseline: 1.1452x; 1.0091x over previous
"""Trainium2 Bass kernel v5 for 16-head causal self-attention (KaplanAttention).

Sharding (8 cores): core c handles batch b = c // 4 and head group g = c % 4
(heads 4g..4g+3); host sums the 4 partial output projections per batch.

v5 scheduling insight: exp on ScalarE (~72us) is 2x slower than the warm-clock
score matmuls it consumes (~29us), while ALL other PE work (projections, AV,
transposes, final projection, ~72us) almost exactly matches exp time. So the
emission interleaves at ~0.5us granularity: between every score matmul of
window st, weave an AV chain of window st-1 or a projection/final group.
PE then stays near-100% dense through the whole kernel, which also keeps the
HAM clock gate at 2.4 GHz (sparse PE re-throttles to 1.2 GHz within ~3.4us).

Other structure (from v2-v4): s-major streaming attention; AV flipped so the
softmax denominator lands per-partition (cheap reciprocal + tensor_scalar);
causal mask as a {0,1} DVE multiply on diagonal blocks; deferred PE
transposes bring output back to [d, s] for the final projection; weights
loaded before x; fp16 partial outputs summed on host in fp32.
"""

import numpy as np

from concourse import bass_utils, mybir, tile
from concourse import bacc

S = 2048
D = 1024
HPC = 4        # heads per core
DK = 64
DC = HPC * DK  # 256 d-columns per core
NCORES = 8
EC = D // 128  # 8 e-chunks
NJT = S // 128  # 16 j-tiles
NST = S // 512  # 4 s-windows of 512

FP16 = mybir.dt.float16
FP32 = mybir.dt.float32


def _build():
    nc = bacc.Bacc("TRN2", target_bir_lowering=False, debug=False)

    xT_d = nc.dram_tensor("xT", [D, S], FP16, kind="ExternalInput")
    wq_d = nc.dram_tensor("wqT", [D, DC], FP16, kind="ExternalInput")
    wk_d = nc.dram_tensor("wkT", [D, DC], FP16, kind="ExternalInput")
    wv_d = nc.dram_tensor("wvT", [D, DC], FP16, kind="ExternalInput")
    wo_d = nc.dram_tensor("woT", [DC, D], FP16, kind="ExternalInput")
    mask_d = nc.dram_tensor("mask", [128, 2 * 128], FP16, kind="ExternalInput")
    id_d = nc.dram_tensor("ident", [128, 128], FP16, kind="ExternalInput")
    out_d = nc.dram_tensor("out", [S, D], FP16, kind="ExternalOutput")

    with tile.TileContext(nc) as tc:
        with (
            tc.tile_pool(name="const", bufs=1) as const,
            tc.tile_pool(name="work", bufs=1) as work,
            tc.tile_pool(name="upool", bufs=2) as upool,
            tc.tile_pool(name="ospool", bufs=10) as ospool,
            tc.tile_pool(name="zpool", bufs=8) as zpool,
            tc.tile_pool(name="obpool", bufs=3) as obpool,
            tc.tile_pool(name="psBig", bufs=2, space="PSUM") as psBig,
            tc.tile_pool(name="psPo", bufs=3, space="PSUM") as psPo,
            tc.tile_pool(name="psPt", bufs=1, space="PSUM") as psPt,
        ):
            # ---- load inputs (weights early so compute starts right away) ----
            xT = const.tile([128, EC, S], FP16)
            wq = const.tile([128, EC, DC], FP16)
            wk = const.tile([128, EC, DC], FP16)
            wv = const.tile([128, EC, DC], FP16)
            wo = const.tile([128, 2, D], FP16)
            maskD = const.tile([128, 2, 128], FP16)
            ident = const.tile([128, 128], FP16)

            def load_x(c, h):
                nc.sync.dma_start(
                    out=xT[:, c, 1024 * h : 1024 * (h + 1)],
                    in_=xT_d[128 * c : 128 * (c + 1), 1024 * h : 1024 * (h + 1)],
                )

            # weights, then the s<1024 half of x (everything windows 0-1 need),
            # then the rest: compute starts ~8us in instead of ~20us
            nc.sync.dma_start(out=wq, in_=wq_d.rearrange("(c p) d -> p c d", p=128))
            nc.sync.dma_start(out=wk, in_=wk_d.rearrange("(c p) d -> p c d", p=128))
            for c in range(EC):
                load_x(c, 0)
            nc.sync.dma_start(out=wv, in_=wv_d.rearrange("(c p) d -> p c d", p=128))
            nc.sync.dma_start(out=wo, in_=wo_d.rearrange("(c p) d -> p c d", p=128))
            nc.sync.dma_start(
                out=maskD, in_=mask_d.rearrange("p (t c) -> p t c", t=2)
            )
            nc.sync.dma_start(out=ident, in_=id_d[:, :])
            for c in range(EC):
                load_x(c, 1)

            QT = work.tile([128, 2, S], FP16)
            KT = work.tile([128, 2, S], FP16)
            V = work.tile([128, NJT, HPC, 65], FP16)
            nc.vector.memset(V[:, :, :, 64:65], 1.0)
            outTn = work.tile([128, 2, S], FP16)  # [d-of-pair, hp, s], normalized

            def proj_qk(w_t, dst, hp, st):
                ps = psBig.tile([128, 2, 512], FP32, tag="big")
                for c in range(EC):
                    nc.tensor.matmul(
                        ps[:, 0, :],
                        w_t[:, c, 128 * hp : 128 * (hp + 1)],
                        xT[:, c, 512 * st : 512 * (st + 1)],
                        start=(c == 0),
                        stop=(c == EC - 1),
                    )
                nc.vector.tensor_copy(
                    out=dst[:, hp, 512 * st : 512 * (st + 1)], in_=ps[:, 0, :]
                )

            def proj_v(jt):
                ps = psBig.tile([128, 2, 512], FP32, tag="big")
                psd = ps[:, 0, 0:DC]
                for c in range(EC):
                    nc.tensor.matmul(
                        psd,
                        xT[:, c, 128 * jt : 128 * (jt + 1)],
                        wv[:, c, :],
                        start=(c == 0),
                        stop=(c == EC - 1),
                    )
                nc.vector.tensor_copy(
                    out=V[:, jt, :, 0:64],
                    in_=psd.rearrange("p (h d) -> p h d", h=HPC),
                )

            # deferred PE transposes: (hp, sb, os_tile)
            pending_t = []

            def emit_transpose(use_act=False):
                hp, sb, os_t = pending_t.pop(0)
                pt = psPt.tile([128, 128], FP16, tag="pt")
                nc.tensor.transpose(pt, os_t, ident)
                dst = outTn[:, hp, 128 * sb : 128 * (sb + 1)]
                if use_act:
                    nc.scalar.copy(out=dst, in_=pt)
                else:
                    nc.vector.tensor_copy(out=dst, in_=pt)

            def emit_final(sb):
                # transposes this final depends on must be emitted first
                while pending_t and (
                    pending_t[0][0] == 0 or pending_t[0][1] <= sb + 1
                ):
                    emit_transpose(use_act=(sb >= 11))
                psf = psBig.tile([128, 2, 512], FP32, tag="big")
                for mt in range(2):
                    for hp in range(2):
                        nc.tensor.matmul(
                            psf[:, mt, :],
                            outTn[:, hp, 128 * sb : 128 * (sb + 1)],
                            wo[:, hp, 512 * mt : 512 * (mt + 1)],
                            start=(hp == 0),
                            stop=(hp == 1),
                        )
                ob = obpool.tile([128, 2, 512], FP16, tag="ob")
                if sb >= 12:
                    nc.scalar.copy(out=ob, in_=psf)
                else:
                    nc.vector.tensor_copy(out=ob, in_=psf)
                nc.sync.dma_start(
                    out=out_d[128 * sb : 128 * (sb + 1), :].rearrange(
                        "p (t c) -> p t c", t=2
                    ),
                    in_=ob,
                )

            def av_chain_thunks(hp, st, Ut):
                """One thunk per s-block: AV chain + normalize + pending push."""

                def chain(sbl):
                    sb = 4 * st + sbl
                    po = psPo.tile([128, 2, 65], FP32, tag="po")
                    for hi in range(2):
                        for k in range(sb + 1):
                            nc.tensor.matmul(
                                po[:, hi, :],
                                Ut[:, hi, k, 128 * sbl : 128 * (sbl + 1)],
                                V[:, k, 2 * hp + hi, :],
                                start=(k == 0),
                                stop=(k == sb),
                            )
                    zr = zpool.tile([128, 2, 1], FP32, tag="zr")
                    nc.vector.reciprocal(out=zr, in_=po[:, :, 64:65])
                    os_t = ospool.tile([128, 2, DK], FP16, tag="os")
                    for hi in range(2):
                        nc.vector.tensor_scalar_mul(
                            os_t[:, hi, :], po[:, hi, 0:64], zr[:, hi, :]
                        )
                    pending_t.append((hp, sb, os_t))
                    thresh = 2 if (hp == 1 and st == 3) else 4
                    while len(pending_t) > thresh:
                        emit_transpose()

                return [lambda sbl=sbl: chain(sbl) for sbl in range(4)]

            def attn_scores(hp, st, work_items):
                """Emit scores+exp for window st, weaving work_items (AV chains
                of the previous window, projections, finals) between score
                matmuls to keep PE dense. Returns this window's AV thunks."""
                Ut = upool.tile([128, 2, NJT, 512], FP16, tag="U")
                njt = 4 * st + 4
                emitted = 0
                for jt in range(njt):
                    off = max(0, 128 * jt - 512 * st)
                    n = 512 - off
                    ps = psBig.tile([128, 2, 512], FP32, tag="big")
                    for hi in range(2):
                        ho = 64 * hi
                        nc.tensor.matmul(
                            ps[:, hi, 0:n],
                            KT[ho : ho + 64, hp, 128 * jt : 128 * (jt + 1)],
                            QT[ho : ho + 64, hp, 512 * st + off : 512 * (st + 1)],
                            start=True,
                            stop=True,
                        )
                    nc.scalar.activation(
                        out=Ut[:, :, jt, off : off + n],
                        in_=ps[:, :, 0:n],
                        func=mybir.ActivationFunctionType.Exp,
                        scale=0.125,
                    )
                    if jt >= 4 * st:  # diagonal 128-block: causal mask
                        nc.vector.tensor_mul(
                            Ut[:, :, jt, off : off + 128],
                            Ut[:, :, jt, off : off + 128],
                            maskD,
                        )
                    # spread work_items evenly across the jt loop
                    want = (len(work_items) + emitted) * (jt + 1) // njt
                    while emitted < want and work_items:
                        work_items.pop(0)()
                        emitted += 1
                while work_items:
                    work_items.pop(0)()
                return av_chain_thunks(hp, st, Ut)

            # ---- phase A: just window 0's projections; the rest is woven ----
            proj_qk(wq, QT, 0, 0)
            proj_qk(wk, KT, 0, 0)
            for jt in range(4):
                proj_v(jt)
            proj_qk(wq, QT, 1, 0)
            proj_qk(wk, KT, 1, 0)

            # ---- attention: hp=0 then hp=1, with cross-window weaving ----
            av_prev = []
            for st in range(NST):
                fills = []
                if st + 1 < NST:
                    nst = st + 1
                    fills = (
                        [
                            lambda nst=nst: proj_qk(wq, QT, 0, nst),
                            lambda nst=nst: proj_qk(wk, KT, 0, nst),
                        ]
                        + [lambda jt=jt: proj_v(jt) for jt in range(4 * nst, 4 * nst + 4)]
                        + [
                            lambda nst=nst: proj_qk(wq, QT, 1, nst),
                            lambda nst=nst: proj_qk(wk, KT, 1, nst),
                        ]
                    )
                av_prev = attn_scores(0, st, av_prev + fills)
            for st in range(NST):
                fills = []
                if st == 2:
                    fills = [lambda sb=sb: emit_final(sb) for sb in range(0, 4)]
                elif st == 3:
                    fills = [lambda sb=sb: emit_final(sb) for sb in range(4, 12)]
                av_prev = attn_scores(1, st, av_prev + fills)
            # tail: last window's AV chains interleaved with remaining finals
            finals = list(range(12, 16))
            for t in av_prev:
                t()
                if finals:
                    emit_final(finals.pop(0))
            while finals:
                emit_final(finals.pop(0))

    nc.compile()
    return nc


_NC = None


def _prep_in_maps(x, W_q, W_k, W_v, W_o):
    x = np.asarray(x, dtype=np.float32)
    W_q = np.asarray(W_q, dtype=np.float32)
    W_k = np.asarray(W_k, dtype=np.float32)
    W_v = np.asarray(W_v, dtype=np.float32)
    W_o = np.asarray(W_o, dtype=np.float32)
    mask01 = np.triu(np.ones((128, 128), dtype=np.float16))
    mask2 = np.concatenate([mask01, mask01], axis=1)
    ident = np.eye(128, dtype=np.float16)
    in_maps = []
    for c in range(NCORES):
        b, g = divmod(c, 4)
        cols = slice(DC * g, DC * (g + 1))
        in_maps.append(
            {
                "xT": np.ascontiguousarray(x[b].T).astype(np.float16),
                "wqT": np.ascontiguousarray(W_q[cols, :].T).astype(np.float16),
                "wkT": np.ascontiguousarray(W_k[cols, :].T).astype(np.float16),
                "wvT": np.ascontiguousarray(W_v[cols, :].T).astype(np.float16),
                "woT": np.ascontiguousarray(W_o[:, cols].T).astype(np.float16),
                "mask": mask2,
                "ident": ident,
            }
        )
    return in_maps


def _run(x, W_q, W_k, W_v, W_o, **spmd_kwargs):
    global _NC
    if _NC is None:
        _NC = _build()
    in_maps = _prep_in_maps(x, W_q, W_k, W_v, W_o)
    res = bass_utils.run_bass_kernel_spmd(
        _NC, in_maps, core_ids=list(range(NCORES)), **spmd_kwargs
    )
    out = np.empty((2, S, D), dtype=np.float32)
    for b in range(2):
        out[b] = (
            res.results[4 * b]["out"].astype(np.float32)
            + res.results[4 * b + 1]["out"].astype(np.float32)
            + res.results[4 * b + 2]["out"].astype(np.float32)
            + res.results[4 * b + 3]["out"].astype(np.float32)
        )
    return out, res


def kernel(x, W_q, W_k, W_v, W_o):
    out, _ = _run(x, W_q, W_k, W_v, W_o)
    return out


# revision 13
# speedup vs baseline: 1.1865x; 1.0361x over previous
"""Trainium2 Bass kernel v5 for 16-head causal self-attention (KaplanAttention).

Sharding (8 cores): core c handles batch b = c // 4 and head group g = c % 4
(heads 4g..4g+3); host sums the 4 partial output projections per batch.

v5 scheduling insight: exp on ScalarE (~72us) is 2x slower than the warm-clock
score matmuls it consumes (~29us), while ALL other PE work (projections, AV,
transposes, final projection, ~72us) almost exactly matches exp time. So the
emission interleaves at ~0.5us granularity: between every score matmul of
window st, weave an AV chain of window st-1 or a projection/final group.
PE then stays near-100% dense through the whole kernel, which also keeps the
HAM clock gate at 2.4 GHz (sparse PE re-throttles to 1.2 GHz within ~3.4us).

Other structure (from v2-v4): s-major streaming attention; AV flipped so the
softmax denominator lands per-partition (cheap reciprocal + tensor_scalar);
causal mask as a {0,1} DVE multiply on diagonal blocks; deferred PE
transposes bring output back to [d, s] for the final projection; weights
loaded before x; fp16 partial outputs summed on host in fp32.
"""

import numpy as np

from concourse import bass_utils, mybir, tile
from concourse import bacc

S = 2048
D = 1024
HPC = 4        # heads per core
DK = 64
DC = HPC * DK  # 256 d-columns per core
NCORES = 8
EC = D // 128  # 8 e-chunks
NJT = S // 128  # 16 j-tiles
NST = S // 512  # 4 s-windows of 512

FP16 = mybir.dt.float16
FP32 = mybir.dt.float32


def _build():
    nc = bacc.Bacc("TRN2", target_bir_lowering=False, debug=False)

    xT_d = nc.dram_tensor("xT", [D, S], FP16, kind="ExternalInput")
    wq_d = nc.dram_tensor("wqT", [D, DC], FP16, kind="ExternalInput")
    wk_d = nc.dram_tensor("wkT", [D, DC], FP16, kind="ExternalInput")
    wv_d = nc.dram_tensor("wvT", [D, DC], FP16, kind="ExternalInput")
    wo_d = nc.dram_tensor("woT", [DC, D], FP16, kind="ExternalInput")
    mask_d = nc.dram_tensor("mask", [128, 2 * 128], FP16, kind="ExternalInput")
    id_d = nc.dram_tensor("ident", [128, 128], FP16, kind="ExternalInput")
    out_d = nc.dram_tensor("out", [S, D], FP16, kind="ExternalOutput")

    with tile.TileContext(nc) as tc:
        with (
            tc.tile_pool(name="const", bufs=1) as const,
            tc.tile_pool(name="work", bufs=1) as work,
            tc.tile_pool(name="upool", bufs=2) as upool,
            tc.tile_pool(name="ospool", bufs=10) as ospool,
            tc.tile_pool(name="zpool", bufs=8) as zpool,
            tc.tile_pool(name="obpool", bufs=3) as obpool,
            tc.tile_pool(name="psBig", bufs=2, space="PSUM") as psBig,
            tc.tile_pool(name="psPo", bufs=3, space="PSUM") as psPo,
            tc.tile_pool(name="psPt", bufs=1, space="PSUM") as psPt,
        ):
            # ---- load inputs (weights early so compute starts right away) ----
            xT = const.tile([128, EC, S], FP16)
            wq = const.tile([128, EC, DC], FP16)
            wk = const.tile([128, EC, DC], FP16)
            wv = const.tile([128, EC, DC], FP16)
            wo = const.tile([128, 2, D], FP16)
            maskD = const.tile([128, 2, 128], FP16)
            ident = const.tile([128, 128], FP16)

            def load_x(c, h):
                nc.sync.dma_start(
                    out=xT[:, c, 1024 * h : 1024 * (h + 1)],
                    in_=xT_d[128 * c : 128 * (c + 1), 1024 * h : 1024 * (h + 1)],
                )

            # weights, then the s<1024 half of x (everything windows 0-1 need),
            # then the rest: compute starts ~8us in instead of ~20us
            nc.sync.dma_start(out=wq, in_=wq_d.rearrange("(c p) d -> p c d", p=128))
            nc.sync.dma_start(out=wk, in_=wk_d.rearrange("(c p) d -> p c d", p=128))
            for c in range(EC):
                load_x(c, 0)
            nc.sync.dma_start(out=wv, in_=wv_d.rearrange("(c p) d -> p c d", p=128))
            nc.sync.dma_start(out=wo, in_=wo_d.rearrange("(c p) d -> p c d", p=128))
            nc.sync.dma_start(
                out=maskD, in_=mask_d.rearrange("p (t c) -> p t c", t=2)
            )
            nc.sync.dma_start(out=ident, in_=id_d[:, :])
            for c in range(EC):
                load_x(c, 1)

            QT = work.tile([128, 2, S], FP16)
            KT = work.tile([128, 2, S], FP16)
            V = work.tile([128, NJT, HPC, 65], FP16)
            nc.vector.memset(V[:, :, :, 64:65], 1.0)
            outTn = work.tile([128, 2, S], FP16)  # [d-of-pair, hp, s], normalized

            def proj_qk(w_t, dst, hp, st):
                ps = psBig.tile([128, 2, 512], FP32, tag="big")
                for c in range(EC):
                    nc.tensor.matmul(
                        ps[:, 0, :],
                        w_t[:, c, 128 * hp : 128 * (hp + 1)],
                        xT[:, c, 512 * st : 512 * (st + 1)],
                        start=(c == 0),
                        stop=(c == EC - 1),
                    )
                nc.vector.tensor_copy(
                    out=dst[:, hp, 512 * st : 512 * (st + 1)], in_=ps[:, 0, :]
                )

            def proj_v(jt):
                ps = psBig.tile([128, 2, 512], FP32, tag="big")
                psd = ps[:, 0, 0:DC]
                for c in range(EC):
                    nc.tensor.matmul(
                        psd,
                        xT[:, c, 128 * jt : 128 * (jt + 1)],
                        wv[:, c, :],
                        start=(c == 0),
                        stop=(c == EC - 1),
                    )
                nc.vector.tensor_copy(
                    out=V[:, jt, :, 0:64],
                    in_=psd.rearrange("p (h d) -> p h d", h=HPC),
                )

            # deferred PE transposes: (hp, sb, os_tile)
            pending_t = []

            def emit_transpose(use_act=False):
                hp, sb, os_t = pending_t.pop(0)
                pt = psPt.tile([128, 128], FP16, tag="pt")
                nc.tensor.transpose(pt, os_t, ident)
                dst = outTn[:, hp, 128 * sb : 128 * (sb + 1)]
                if use_act:
                    nc.scalar.copy(out=dst, in_=pt)
                else:
                    nc.vector.tensor_copy(out=dst, in_=pt)

            def emit_final(sb):
                # transposes this final depends on must be emitted first
                while pending_t and (
                    pending_t[0][0] == 0 or pending_t[0][1] <= sb + 1
                ):
                    emit_transpose(use_act=(sb >= 11))
                psf = psBig.tile([128, 2, 512], FP32, tag="big")
                for mt in range(2):
                    for hp in range(2):
                        nc.tensor.matmul(
                            psf[:, mt, :],
                            outTn[:, hp, 128 * sb : 128 * (sb + 1)],
                            wo[:, hp, 512 * mt : 512 * (mt + 1)],
                            start=(hp == 0),
                            stop=(hp == 1),
                        )
                ob = obpool.tile([128, 2, 512], FP16, tag="ob")
                if sb >= 12:
                    nc.scalar.copy(out=ob, in_=psf)
                else:
                    nc.vector.tensor_copy(out=ob, in_=psf)
                nc.sync.dma_start(
                    out=out_d[128 * sb : 128 * (sb + 1), :].rearrange(
                        "p (t c) -> p t c", t=2
                    ),
                    in_=ob,
                )

            def av_chain_thunks(hp, st, Ut):
                """One thunk per s-block: AV chain + normalize + pending push."""

                def chain(sbl):
                    sb = 4 * st + sbl
                    po = psPo.tile([128, 2, 65], FP32, tag="po")
                    for hi in range(2):
                        for k in range(sb + 1):
                            nc.tensor.matmul(
                                po[:, hi, :],
                                Ut[:, hi, k, 128 * sbl : 128 * (sbl + 1)],
                                V[:, k, 2 * hp + hi, :],
                                start=(k == 0),
                                stop=(k == sb),
                            )
                    zr = zpool.tile([128, 2, 1], FP32, tag="zr")
                    nc.vector.reciprocal(out=zr, in_=po[:, :, 64:65])
                    os_t = ospool.tile([128, 2, DK], FP16, tag="os")
                    for hi in range(2):
                        nc.vector.tensor_scalar_mul(
                            os_t[:, hi, :], po[:, hi, 0:64], zr[:, hi, :]
                        )
                    pending_t.append((hp, sb, os_t))
                    thresh = 2 if (hp == 1 and st == 3) else 4
                    while len(pending_t) > thresh:
                        emit_transpose()

                return [lambda sbl=sbl: chain(sbl) for sbl in range(4)]

            def attn_scores(hp, st, work_items):
                """Emit scores+exp for window st, weaving work_items (AV chains
                of the previous window, projections, finals) between score
                matmuls to keep PE dense. Returns this window's AV thunks."""
                Ut = upool.tile([128, 2, NJT, 512], FP16, tag="U")
                njt = 4 * st + 4
                emitted = 0
                for jt in range(njt):
                    off = max(0, 128 * jt - 512 * st)
                    n = 512 - off
                    ps = psBig.tile([128, 2, 512], FP32, tag="big")
                    for hi in range(2):
                        ho = 64 * hi
                        nc.tensor.matmul(
                            ps[:, hi, 0:n],
                            KT[ho : ho + 64, hp, 128 * jt : 128 * (jt + 1)],
                            QT[ho : ho + 64, hp, 512 * st + off : 512 * (st + 1)],
                            start=True,
                            stop=True,
                        )
                    nc.scalar.activation(
                        out=Ut[:, :, jt, off : off + n],
                        in_=ps[:, :, 0:n],
                        func=mybir.ActivationFunctionType.Exp,
                        scale=0.125,
                    )
                    if jt >= 4 * st:  # diagonal 128-block: causal mask
                        nc.vector.tensor_mul(
                            Ut[:, :, jt, off : off + 128],
                            Ut[:, :, jt, off : off + 128],
                            maskD,
                        )
                    # spread work_items evenly across the jt loop
                    want = (len(work_items) + emitted) * (jt + 1) // njt
                    while emitted < want and work_items:
                        work_items.pop(0)()
                        emitted += 1
                while work_items:
                    work_items.pop(0)()
                return av_chain_thunks(hp, st, Ut)

            # ---- phase A: just window 0's hp0 projections; rest is woven ----
            proj_qk(wq, QT, 0, 0)
            proj_qk(wk, KT, 0, 0)

            # fill schedules balanced so each window's woven PE work matches
            # its exp time (hp0-st3 and hp1-st1 were starved before)
            def f_qk0(st):
                return [
                    lambda st=st: proj_qk(wq, QT, 0, st),
                    lambda st=st: proj_qk(wk, KT, 0, st),
                ]

            def f_qk1(st):
                return [
                    lambda st=st: proj_qk(wq, QT, 1, st),
                    lambda st=st: proj_qk(wk, KT, 1, st),
                ]

            def f_v(st):
                return [lambda jt=jt: proj_v(jt) for jt in range(4 * st, 4 * st + 4)]

            fills0 = {
                0: f_qk0(1) + f_v(0),
                1: f_qk0(2) + f_v(1) + f_qk1(0),
                2: f_qk0(3) + f_v(2) + f_qk1(1),
                3: f_v(3),
            }
            fills1 = {
                1: f_qk1(2) + f_qk1(3),
                2: [lambda sb=sb: emit_final(sb) for sb in range(0, 4)],
                3: [lambda sb=sb: emit_final(sb) for sb in range(4, 12)],
            }

            # ---- attention: hp=0 then hp=1, with cross-window weaving ----
            av_prev = []
            for st in range(NST):
                av_prev = attn_scores(0, st, av_prev + fills0.get(st, []))
            for st in range(NST):
                av_prev = attn_scores(1, st, av_prev + fills1.get(st, []))
            # tail: last window's AV chains interleaved with remaining finals
            finals = list(range(12, 16))
            for t in av_prev:
                t()
                if finals:
                    emit_final(finals.pop(0))
            while finals:
                emit_final(finals.pop(0))

    nc.compile()
    return nc


_NC = None


def _prep_in_maps(x, W_q, W_k, W_v, W_o):
    x = np.asarray(x, dtype=np.float32)
    W_q = np.asarray(W_q, dtype=np.float32)
    W_k = np.asarray(W_k, dtype=np.float32)
    W_v = np.asarray(W_v, dtype=np.float32)
    W_o = np.asarray(W_o, dtype=np.float32)
    mask01 = np.triu(np.ones((128, 128), dtype=np.float16))
    mask2 = np.concatenate([mask01, mask01], axis=1)
    ident = np.eye(128, dtype=np.float16)
    in_maps = []
    for c in range(NCORES):
        b, g = divmod(c, 4)
        cols = slice(DC * g, DC * (g + 1))
        in_maps.append(
            {
                "xT": np.ascontiguousarray(x[b].T).astype(np.float16),
                "wqT": np.ascontiguousarray(W_q[cols, :].T).astype(np.float16),
                "wkT": np.ascontiguousarray(W_k[cols, :].T).astype(np.float16),
                "wvT": np.ascontiguousarray(W_v[cols, :].T).astype(np.float16),
                "woT": np.ascontiguousarray(W_o[:, cols].T).astype(np.float16),
                "mask": mask2,
                "ident": ident,
            }
        )
    return in_maps


def _run(x, W_q, W_k, W_v, W_o, **spmd_kwargs):
    global _NC
    if _NC is None:
        _NC = _build()
    in_maps = _prep_in_maps(x, W_q, W_k, W_v, W_o)
    res = bass_utils.run_bass_kernel_spmd(
        _NC, in_maps, core_ids=list(range(NCORES)), **spmd_kwargs
    )
    out = np.empty((2, S, D), dtype=np.float32)
    for b in range(2):
        out[b] = (
            res.results[4 * b]["out"].astype(np.float32)
            + res.results[4 * b + 1]["out"].astype(np.float32)
            + res.results[4 * b + 2]["out"].astype(np.float32)
            + res.results[4 * b + 3]["out"].astype(np.float32)
        )
    return out, res


def kernel(x, W_q, W_k, W_v, W_o):
    out, _ = _run(x, W_q, W_k, W_v, W_o)
    return out


# revision 14
# speedup vs baseline: 1.1931x; 1.0056x over previous
"""Trainium2 Bass kernel v5 for 16-head causal self-attention (KaplanAttention).

Sharding (8 cores): core c handles batch b = c // 4 and head group g = c % 4
(heads 4g..4g+3); host sums the 4 partial output projections per batch.

v5 scheduling insight: exp on ScalarE (~72us) is 2x slower than the warm-clock
score matmuls it consumes (~29us), while ALL other PE work (projections, AV,
transposes, final projection, ~72us) almost exactly matches exp time. So the
emission interleaves at ~0.5us granularity: between every score matmul of
window st, weave an AV chain of window st-1 or a projection/final group.
PE then stays near-100% dense through the whole kernel, which also keeps the
HAM clock gate at 2.4 GHz (sparse PE re-throttles to 1.2 GHz within ~3.4us).

Other structure (from v2-v4): s-major streaming attention; AV flipped so the
softmax denominator lands per-partition (cheap reciprocal + tensor_scalar);
causal mask as a {0,1} DVE multiply on diagonal blocks; deferred PE
transposes bring output back to [d, s] for the final projection; weights
loaded before x; fp16 partial outputs summed on host in fp32.
"""

import numpy as np

from concourse import bass_utils, mybir, tile
from concourse import bacc

S = 2048
D = 1024
HPC = 4        # heads per core
DK = 64
DC = HPC * DK  # 256 d-columns per core
NCORES = 8
EC = D // 128  # 8 e-chunks
NJT = S // 128  # 16 j-tiles
NST = S // 512  # 4 s-windows of 512

FP16 = mybir.dt.float16
FP32 = mybir.dt.float32


def _build():
    nc = bacc.Bacc("TRN2", target_bir_lowering=False, debug=False)

    xT_d = nc.dram_tensor("xT", [D, S], FP16, kind="ExternalInput")
    wq_d = nc.dram_tensor("wqT", [D, DC], FP16, kind="ExternalInput")
    wk_d = nc.dram_tensor("wkT", [D, DC], FP16, kind="ExternalInput")
    wv_d = nc.dram_tensor("wvT", [D, DC], FP16, kind="ExternalInput")
    wo_d = nc.dram_tensor("woT", [DC, D], FP16, kind="ExternalInput")
    mask_d = nc.dram_tensor("mask", [128, 2 * 128], FP16, kind="ExternalInput")
    id_d = nc.dram_tensor("ident", [128, 128], FP16, kind="ExternalInput")
    out_d = nc.dram_tensor("out", [S, D], FP16, kind="ExternalOutput")

    with tile.TileContext(nc) as tc:
        with (
            tc.tile_pool(name="const", bufs=1) as const,
            tc.tile_pool(name="work", bufs=1) as work,
            tc.tile_pool(name="upool", bufs=2) as upool,
            tc.tile_pool(name="ospool", bufs=10) as ospool,
            tc.tile_pool(name="zpool", bufs=8) as zpool,
            tc.tile_pool(name="obpool", bufs=3) as obpool,
            tc.tile_pool(name="psBig", bufs=2, space="PSUM") as psBig,
            tc.tile_pool(name="psPo", bufs=3, space="PSUM") as psPo,
            tc.tile_pool(name="psPt", bufs=1, space="PSUM") as psPt,
        ):
            # ---- load inputs (weights early so compute starts right away) ----
            xT = const.tile([128, EC, S], FP16)
            wq = const.tile([128, EC, DC], FP16)
            wk = const.tile([128, EC, DC], FP16)
            wv = const.tile([128, EC, DC], FP16)
            wo = const.tile([128, 2, D], FP16)
            maskD = const.tile([128, 2, 128], FP16)
            ident = const.tile([128, 128], FP16)

            def load_x(c, h):
                nc.sync.dma_start(
                    out=xT[:, c, 1024 * h : 1024 * (h + 1)],
                    in_=xT_d[128 * c : 128 * (c + 1), 1024 * h : 1024 * (h + 1)],
                )

            # weights, then the s<1024 half of x (everything windows 0-1 need),
            # then the rest: compute starts ~8us in instead of ~20us
            nc.sync.dma_start(out=wq, in_=wq_d.rearrange("(c p) d -> p c d", p=128))
            nc.sync.dma_start(out=wk, in_=wk_d.rearrange("(c p) d -> p c d", p=128))
            for c in range(EC):
                load_x(c, 0)
            nc.sync.dma_start(out=wv, in_=wv_d.rearrange("(c p) d -> p c d", p=128))
            nc.sync.dma_start(out=wo, in_=wo_d.rearrange("(c p) d -> p c d", p=128))
            nc.sync.dma_start(
                out=maskD, in_=mask_d.rearrange("p (t c) -> p t c", t=2)
            )
            nc.sync.dma_start(out=ident, in_=id_d[:, :])
            for c in range(EC):
                load_x(c, 1)

            QT = work.tile([128, 2, S], FP16)
            KT = work.tile([128, 2, S], FP16)
            V = work.tile([128, NJT, HPC, 65], FP16)
            nc.vector.memset(V[:, :, :, 64:65], 1.0)
            outTn = work.tile([128, 2, S], FP16)  # [d-of-pair, hp, s], normalized

            def proj_qk(w_t, dst, hp, st):
                ps = psBig.tile([128, 2, 512], FP32, tag="big")
                for c in range(EC):
                    nc.tensor.matmul(
                        ps[:, 0, :],
                        w_t[:, c, 128 * hp : 128 * (hp + 1)],
                        xT[:, c, 512 * st : 512 * (st + 1)],
                        start=(c == 0),
                        stop=(c == EC - 1),
                    )
                nc.vector.tensor_copy(
                    out=dst[:, hp, 512 * st : 512 * (st + 1)], in_=ps[:, 0, :]
                )

            def proj_v(jt):
                ps = psBig.tile([128, 2, 512], FP32, tag="big")
                psd = ps[:, 0, 0:DC]
                for c in range(EC):
                    nc.tensor.matmul(
                        psd,
                        xT[:, c, 128 * jt : 128 * (jt + 1)],
                        wv[:, c, :],
                        start=(c == 0),
                        stop=(c == EC - 1),
                    )
                vdst = V[:, jt, :, 0:64]
                vsrc = psd.rearrange("p (h d) -> p h d", h=HPC)
                if jt < 8:
                    nc.scalar.copy(out=vdst, in_=vsrc)
                else:
                    nc.vector.tensor_copy(out=vdst, in_=vsrc)

            # deferred PE transposes: (hp, sb, os_tile)
            pending_t = []

            def emit_transpose(use_act=False):
                hp, sb, os_t = pending_t.pop(0)
                pt = psPt.tile([128, 128], FP16, tag="pt")
                nc.tensor.transpose(pt, os_t, ident)
                dst = outTn[:, hp, 128 * sb : 128 * (sb + 1)]
                if use_act:
                    nc.scalar.copy(out=dst, in_=pt)
                else:
                    nc.vector.tensor_copy(out=dst, in_=pt)

            def emit_final(sb):
                # transposes this final depends on must be emitted first
                while pending_t and (
                    pending_t[0][0] == 0 or pending_t[0][1] <= sb + 1
                ):
                    emit_transpose(use_act=(sb >= 11))
                psf = psBig.tile([128, 2, 512], FP32, tag="big")
                for mt in range(2):
                    for hp in range(2):
                        nc.tensor.matmul(
                            psf[:, mt, :],
                            outTn[:, hp, 128 * sb : 128 * (sb + 1)],
                            wo[:, hp, 512 * mt : 512 * (mt + 1)],
                            start=(hp == 0),
                            stop=(hp == 1),
                        )
                ob = obpool.tile([128, 2, 512], FP16, tag="ob")
                if sb >= 12:
                    nc.vector.tensor_copy(out=ob[:, 0, :], in_=psf[:, 0, :])
                    nc.scalar.copy(out=ob[:, 1, :], in_=psf[:, 1, :])
                else:
                    nc.vector.tensor_copy(out=ob, in_=psf)
                nc.sync.dma_start(
                    out=out_d[128 * sb : 128 * (sb + 1), :].rearrange(
                        "p (t c) -> p t c", t=2
                    ),
                    in_=ob,
                )

            def av_chain_thunks(hp, st, Ut):
                """One thunk per s-block: AV chain + normalize + pending push."""

                def chain(sbl):
                    sb = 4 * st + sbl
                    po = psPo.tile([128, 2, 65], FP32, tag="po")
                    for hi in range(2):
                        for k in range(sb + 1):
                            nc.tensor.matmul(
                                po[:, hi, :],
                                Ut[:, hi, k, 128 * sbl : 128 * (sbl + 1)],
                                V[:, k, 2 * hp + hi, :],
                                start=(k == 0),
                                stop=(k == sb),
                            )
                    zr = zpool.tile([128, 2, 1], FP32, tag="zr")
                    nc.vector.reciprocal(out=zr, in_=po[:, :, 64:65])
                    os_t = ospool.tile([128, 2, DK], FP16, tag="os")
                    for hi in range(2):
                        nc.vector.tensor_scalar_mul(
                            os_t[:, hi, :], po[:, hi, 0:64], zr[:, hi, :]
                        )
                    pending_t.append((hp, sb, os_t))
                    thresh = 2 if (hp == 1 and st == 3) else 4
                    while len(pending_t) > thresh:
                        emit_transpose()

                return [lambda sbl=sbl: chain(sbl) for sbl in range(4)]

            def attn_scores(hp, st, work_items):
                """Emit scores+exp for window st, weaving work_items (AV chains
                of the previous window, projections, finals) between score
                matmuls to keep PE dense. Returns this window's AV thunks."""
                Ut = upool.tile([128, 2, NJT, 512], FP16, tag="U")
                njt = 4 * st + 4
                emitted = 0
                for jt in range(njt):
                    off = max(0, 128 * jt - 512 * st)
                    n = 512 - off
                    ps = psBig.tile([128, 2, 512], FP32, tag="big")
                    for hi in range(2):
                        ho = 64 * hi
                        nc.tensor.matmul(
                            ps[:, hi, 0:n],
                            KT[ho : ho + 64, hp, 128 * jt : 128 * (jt + 1)],
                            QT[ho : ho + 64, hp, 512 * st + off : 512 * (st + 1)],
                            start=True,
                            stop=True,
                        )
                    nc.scalar.activation(
                        out=Ut[:, :, jt, off : off + n],
                        in_=ps[:, :, 0:n],
                        func=mybir.ActivationFunctionType.Exp,
                        scale=0.125,
                    )
                    if jt >= 4 * st:  # diagonal 128-block: causal mask
                        nc.vector.tensor_mul(
                            Ut[:, :, jt, off : off + 128],
                            Ut[:, :, jt, off : off + 128],
                            maskD,
                        )
                    # spread work_items evenly across the jt loop
                    want = (len(work_items) + emitted) * (jt + 1) // njt
                    while emitted < want and work_items:
                        work_items.pop(0)()
                        emitted += 1
                while work_items:
                    work_items.pop(0)()
                return av_chain_thunks(hp, st, Ut)

            # ---- phase A: window 0's hp0 Q/K, c-major interleaved in one
            # 2-bank PSUM tile so both finish right at the last x chunk ----
            psA = psBig.tile([128, 2, 512], FP32, tag="big")
            for c in range(EC):
                nc.tensor.matmul(
                    psA[:, 0, :], wq[:, c, 0:128], xT[:, c, 0:512],
                    start=(c == 0), stop=(c == EC - 1),
                )
                nc.tensor.matmul(
                    psA[:, 1, :], wk[:, c, 0:128], xT[:, c, 0:512],
                    start=(c == 0), stop=(c == EC - 1),
                )
            nc.vector.tensor_copy(out=QT[:, 0, 0:512], in_=psA[:, 0, :])
            nc.vector.tensor_copy(out=KT[:, 0, 0:512], in_=psA[:, 1, :])

            # fill schedules balanced so each window's woven PE work matches
            # its exp time (hp0-st3 and hp1-st1 were starved before)
            def f_qk0(st):
                return [
                    lambda st=st: proj_qk(wq, QT, 0, st),
                    lambda st=st: proj_qk(wk, KT, 0, st),
                ]

            def f_qk1(st):
                return [
                    lambda st=st: proj_qk(wq, QT, 1, st),
                    lambda st=st: proj_qk(wk, KT, 1, st),
                ]

            def f_v(st):
                return [lambda jt=jt: proj_v(jt) for jt in range(4 * st, 4 * st + 4)]

            fills0 = {
                0: f_qk0(1) + f_v(0),
                1: f_qk0(2) + f_v(1) + f_qk1(0),
                2: f_qk0(3) + f_v(2) + f_qk1(1),
                3: f_v(3),
            }
            fills1 = {
                1: f_qk1(2) + f_qk1(3),
                2: [lambda sb=sb: emit_final(sb) for sb in range(0, 4)],
                3: [lambda sb=sb: emit_final(sb) for sb in range(4, 12)],
            }

            # ---- attention: hp=0 then hp=1, with cross-window weaving ----
            av_prev = []
            for st in range(NST):
                av_prev = attn_scores(0, st, av_prev + fills0.get(st, []))
            for st in range(NST):
                av_prev = attn_scores(1, st, av_prev + fills1.get(st, []))
            # tail: last window's AV chains interleaved with remaining finals
            finals = list(range(12, 16))
            for t in av_prev:
                t()
                if finals:
                    emit_final(finals.pop(0))
            while finals:
                emit_final(finals.pop(0))

    nc.compile()
    return nc


_NC = None


def _prep_in_maps(x, W_q, W_k, W_v, W_o):
    x = np.asarray(x, dtype=np.float32)
    W_q = np.asarray(W_q, dtype=np.float32)
    W_k = np.asarray(W_k, dtype=np.float32)
    W_v = np.asarray(W_v, dtype=np.float32)
    W_o = np.asarray(W_o, dtype=np.float32)
    mask01 = np.triu(np.ones((128, 128), dtype=np.float16))
    mask2 = np.concatenate([mask01, mask01], axis=1)
    ident = np.eye(128, dtype=np.float16)
    in_maps = []
    for c in range(NCORES):
        b, g = divmod(c, 4)
        cols = slice(DC * g, DC * (g + 1))
        in_maps.append(
            {
                "xT": np.ascontiguousarray(x[b].T).astype(np.float16),
                "wqT": np.ascontiguousarray(W_q[cols, :].T).astype(np.float16),
                "wkT": np.ascontiguousarray(W_k[cols, :].T).astype(np.float16),
                "wvT": np.ascontiguousarray(W_v[cols, :].T).astype(np.float16),
                "woT": np.ascontiguousarray(W_o[:, cols].T).astype(np.float16),
                "mask": mask2,
                "ident": ident,
            }
        )
    return in_maps


def _run(x, W_q, W_k, W_v, W_o, **spmd_kwargs):
    global _NC
    if _NC is None:
        _NC = _build()
    in_maps = _prep_in_maps(x, W_q, W_k, W_v, W_o)
    res = bass_utils.run_bass_kernel_spmd(
        _NC, in_maps, core_ids=list(range(NCORES)), **spmd_kwargs
    )
    out = np.empty((2, S, D), dtype=np.float32)
    for b in range(2):
        out[b] = (
            res.results[4 * b]["out"].astype(np.float32)
            + res.results[4 * b + 1]["out"].astype(np.float32)
            + res.results[4 * b + 2]["out"].astype(np.float32)
            + res.results[4 * b + 3]["out"].astype(np.float32)
        )
    return out, res


def kernel(x, W_q, W_k, W_v, W_o):
    out, _ = _run(x, W_q, W_k, W_v, W_o)
    return out
